# revision 2
# baseline (speedup 1.0000x reference)
"""Trainium2 Bass kernel for nn_AttLayer (B=32, S=1024, D=1024, 8 NeuronCores).

Computation (per reference):
    qkv    = text @ W.T + b                      [B, S, D]
    scores = (qkv @ qkv^T per sample) / sqrt(D)  [B, S, S]
    attn   = softmax(scores, axis=0)             (softmax over the BATCH dim)
    out    = attn @ qkv                          [B, S, D]

Strategy: data-parallel over batch (4 samples per core). The batch softmax
only couples cores through T[q,k] = sum_b exp(scores[b,q,k]), a [S,S] f32
AllReduce (4 MB). No max subtraction is needed: scores <= ~40 so exp stays
comfortably inside f32 range.

Key implementation points:
 - matmuls run in float32r (TF32-like, full PE rate at N=512, ~1.5e-4 rel err)
   for qkv/scores; bf16 for the attn@qkv output matmul.
 - scores/attn are bitwise symmetric per sample, so attn rows indexed by k
   serve directly as the transposed stationary operand of the final matmul.
 - text^T and W^T are built with TensorE transposes (PE is the only cheap
   transpose path for f32).
 - E = exp(scores/32) (bf16) and qkv (bf16) spill to DRAM between the two
   phases; P = sum_local_b E accumulates in SBUF and is AllReduced via DRAM
   bounce buffers.
"""
import sys

sys.path.insert(0, "/opt/trn_rl_repo")

import numpy as np

import concourse.bacc as bacc
import concourse.mybir as mybir
import concourse.tile as tile
from concourse import masks
from concourse.bass_utils import run_bass_kernel_spmd

F32 = mybir.dt.float32
F32R = mybir.dt.float32r
BF16 = mybir.dt.bfloat16
EXP = mybir.ActivationFunctionType.Exp
COPY = mybir.ActivationFunctionType.Copy
IDENT = mybir.ActivationFunctionType.Identity

N_CORES = 8
B, S, D = 32, 1024, 1024
BL = B // N_CORES          # 4 local samples per core
NT = S // 128              # 8 partition tiles
SCALE = 1.0 / float(np.sqrt(D))

_nc_cache = {}


def _transpose_matrix(nc, ps_pool, src_slab, i, dst, dst_dtype_copy_engine, ident,
                      bias_col=None):
    """Transpose one [128, 1024] slab (rows i*128..) into dst[:, j, i*128:(i+1)*128]
    for j in 0..7, via TensorE transposes packed 4-per-PSUM-bank."""
    for j4 in range(0, NT, 4):
        pt = ps_pool.tile([128, 512], F32, tag="tr", bufs=3)
        for jj in range(4):
            j = j4 + jj
            nc.tensor.transpose(
                pt[:, jj * 128:(jj + 1) * 128],
                src_slab[:, j * 128:(j + 1) * 128],
                ident[:],
            )
        # one strided copy: psum [128, 4, 128] -> dst[:, j4:j4+4, i*128:(i+1)*128]
        dst_ap = dst[:, j4:j4 + 4, i * 128:(i + 1) * 128]
        src_ap = pt[:].rearrange("p (t c) -> p t c", t=4)
        dst_dtype_copy_engine(dst_ap, src_ap)


def _build():
    nc = bacc.Bacc("TRN2", target_bir_lowering=False, debug=False,
                   num_devices=N_CORES)
    text = nc.dram_tensor("text", [BL, S, D], F32, kind="ExternalInput")
    W = nc.dram_tensor("W", [D, D], F32, kind="ExternalInput")
    bias = nc.dram_tensor("b", [D], F32, kind="ExternalInput")
    out = nc.dram_tensor("out", [BL, S, D], F32, kind="ExternalOutput")

    with tile.TileContext(nc) as tc:
        with (
            tc.tile_pool(name="outer", bufs=1) as outer,
            tc.tile_pool(name="dram", bufs=1, space="DRAM") as dram,
        ):
            # persistent DRAM spills / bounce buffers
            e_sp = dram.tile([BL, NT, 128, S], BF16)    # exp(scores) rows by q-tile
            q_sp = dram.tile([BL, NT, 128, D], BF16)    # qkv natural rows by s-tile
            p_bnc = dram.tile([128, NT * S], F32)
            t_bnc = dram.tile([128, NT * S], F32)

            P = outer.tile([128, NT, S], F32)           # sum_b exp, rows by q-tile
            ident = outer.tile([128, 128], F32)
            masks.make_identity(nc, ident[:])
            b_sb = outer.tile([128, NT], F32)
            nc.sync.dma_start(b_sb[:], bias.ap().rearrange("(t p) -> p t", p=128))

            # ---------------- phase 1 ----------------
            with (
                tc.tile_pool(name="ph1", bufs=1) as ph1,
                tc.tile_pool(name="ph1s", bufs=3) as ph1s,
                tc.tile_pool(name="ph1ps", bufs=1, space="PSUM") as pps,
            ):
                def copy_act(dst_ap, src_ap, bcol=None):
                    if bcol is None:
                        nc.scalar.activation(dst_ap, src_ap, COPY)
                    else:
                        nc.scalar.activation(dst_ap, src_ap, IDENT, bias=bcol)

                def copy_dve(dst_ap, src_ap):
                    nc.vector.tensor_copy(dst_ap, src_ap)

                # W^T in f32r: WT[p=d', j, d] = W[d, j*128+p]
                WT = ph1.tile([128, NT, D], F32R, tag="WT")
                for i in range(NT):
                    wslab = ph1s.tile([128, D], F32, tag="slab")
                    nc.sync.dma_start(wslab[:], W.ap()[i * 128:(i + 1) * 128, :])
                    _transpose_matrix(nc, pps, wslab, i, WT, copy_act, ident)

                textT = ph1.tile([128, NT, S], F32R, tag="textT")
                qkvT = ph1.tile([128, NT, S], F32R, tag="qkvT")

                for b in range(BL):
                    # text_b^T in f32r (DVE evacuates transpose PSUM)
                    for i in range(NT):
                        tslab = ph1s.tile([128, D], F32, tag="slab")
                        nc.sync.dma_start(tslab[:],
                                          text.ap()[b, i * 128:(i + 1) * 128, :])
                        _transpose_matrix(nc, pps, tslab, i, textT, copy_dve, ident)

                    # qkvT[d, s] = sum_d' W[d, d'] * text[s, d']  (+ b[d])
                    for dt in range(NT):
                        for sc in range(2):
                            pq = pps.tile([128, 512], F32, tag="mmq", bufs=2)
                            for kt in range(NT):
                                nc.tensor.matmul(
                                    pq[:],
                                    WT[:, kt, dt * 128:(dt + 1) * 128],
                                    textT[:, kt, sc * 512:(sc + 1) * 512],
                                    start=(kt == 0),
                                    stop=(kt == NT - 1),
                                )
                            # add bias, round to f32r
                            copy_act(qkvT[:, dt, sc * 512:(sc + 1) * 512], pq[:],
                                     bcol=b_sb[:, dt:dt + 1])

                    # qkv natural (bf16) via PE transposes, spilled to DRAM
                    for st in range(NT):
                        qstage = ph1s.tile([128, D], BF16, tag="qstage")
                        for d4 in range(0, NT, 4):
                            pt = pps.tile([128, 512], F32, tag="tr", bufs=3)
                            for jj in range(4):
                                dt = d4 + jj
                                nc.tensor.transpose(
                                    pt[:, jj * 128:(jj + 1) * 128],
                                    qkvT[:, dt, st * 128:(st + 1) * 128].bitcast(F32),
                                    ident[:],
                                )
                            nc.vector.tensor_copy(
                                qstage[:, d4 * 128:(d4 + 4) * 128], pt[:])
                        nc.sync.dma_start(q_sp[b, st], qstage[:])

                    # scores + exp + P accumulation + E spill
                    for qt in range(NT):
                        estage = ph1s.tile([128, S], BF16, tag="estage")
                        for kc in range(2):
                            psc = pps.tile([128, 512], F32, tag="mms", bufs=2)
                            for dt in range(NT):
                                nc.tensor.matmul(
                                    psc[:],
                                    qkvT[:, dt, qt * 128:(qt + 1) * 128],
                                    qkvT[:, dt, kc * 512:(kc + 1) * 512],
                                    start=(dt == 0),
                                    stop=(dt == NT - 1),
                                )
                            nc.scalar.activation(
                                estage[:, kc * 512:(kc + 1) * 512], psc[:],
                                EXP, scale=float(SCALE))
                        if b == 0:
                            nc.vector.tensor_copy(P[:, qt, :], estage[:])
                        else:
                            nc.vector.tensor_add(P[:, qt, :], P[:, qt, :], estage[:])
                        nc.sync.dma_start(e_sp[b, qt], estage[:])

            # ---------------- all-reduce of P ----------------
            nc.sync.dma_start(p_bnc[:], P[:].rearrange("p t s -> p (t s)"))
            nc.gpsimd.collective_compute(
                "AllReduce",
                mybir.AluOpType.add,
                replica_groups=[list(range(N_CORES))],
                ins=[p_bnc[:].opt()],
                outs=[t_bnc[:].opt()],
            )

            # ---------------- phase 2 ----------------
            with (
                tc.tile_pool(name="ph2", bufs=1) as ph2,
                tc.tile_pool(name="ph2s", bufs=2) as ph2s,
                tc.tile_pool(name="ph2ps", bufs=1, space="PSUM") as pps2,
            ):
                R = ph2.tile([128, NT, S], F32, tag="R")
                for qt in range(NT):
                    tstage = ph2s.tile([128, S], F32, tag="tstage")
                    nc.sync.dma_start(tstage[:],
                                      t_bnc[:, qt * S:(qt + 1) * S])
                    nc.vector.reciprocal_approx_fast(R[:, qt, :], tstage[:])

                for b in range(BL):
                    e_b = ph2s.tile([128, NT, S], BF16, tag="e_b")
                    qkv_b = ph2s.tile([128, NT, D], BF16, tag="qkv_b")
                    attn_b = ph2s.tile([128, NT, S], BF16, tag="attn_b")
                    nc.sync.dma_start(
                        e_b[:], e_sp[b].rearrange("t p s -> p t s"))
                    nc.sync.dma_start(
                        qkv_b[:], q_sp[b].rearrange("t p s -> p t s"))
                    for qt in range(NT):
                        nc.vector.tensor_mul(attn_b[:, qt, :], e_b[:, qt, :],
                                             R[:, qt, :])
                    # out[q, d] = sum_k attn[q, k] qkv[k, d]; attn is symmetric,
                    # so rows of attn_b indexed by k give lhsT[k, q] directly.
                    for qt in range(NT):
                        ostage = ph2s.tile([128, D], F32, tag="ostage")
                        for dc in range(2):
                            po = pps2.tile([128, 512], F32, tag="mmo", bufs=4)
                            for kt in range(NT):
                                nc.tensor.matmul(
                                    po[:],
                                    attn_b[:, kt, qt * 128:(qt + 1) * 128],
                                    qkv_b[:, kt, dc * 512:(dc + 1) * 512],
                                    start=(kt == 0),
                                    stop=(kt == NT - 1),
                                )
                            nc.scalar.activation(
                                ostage[:, dc * 512:(dc + 1) * 512], po[:], COPY)
                        nc.sync.dma_start(
                            out.ap()[b, qt * 128:(qt + 1) * 128, :], ostage[:])

    nc.compile()
    return nc


def _get_nc():
    if "nc" not in _nc_cache:
        _nc_cache["nc"] = _build()
    return _nc_cache["nc"]


def _run(text, W, b, trace=False):
    text = np.ascontiguousarray(text, dtype=np.float32)
    W = np.ascontiguousarray(W, dtype=np.float32)
    b = np.ascontiguousarray(b, dtype=np.float32)
    shards = np.split(text, N_CORES, axis=0)
    in_maps = [{"text": shards[i], "W": W, "b": b} for i in range(N_CORES)]
    nc = _get_nc()
    res = run_bass_kernel_spmd(nc, in_maps, core_ids=list(range(N_CORES)),
                               trace=trace)
    full = np.concatenate([res.results[i]["out"] for i in range(N_CORES)], axis=0)
    return full, res


def kernel(text, W, b):
    full, _ = _run(text, W, b, trace=False)
    return full
